# revision 31
# baseline (speedup 1.0000x reference)
"""Multi-head attention (QKV proj + RoPE + SDPA + o_proj) on 8 TRN2 cores.

Sharding: DP2 x TP4. Core c handles batch c//4 and heads 4*(c%4)..4*(c%4)+3.
Each core computes its 4 heads' attention and a partial o_proj output
[L, D]; the host sums the 4 partials per batch (row-parallel o_proj).

All matmul operands are bf16 (full-rate on the PE, half the DMA/SBUF of
fp32); accumulation stays fp32 in PSUM. rel-err vs the fp32 reference is
~1e-2, within the 2e-2 gate (fp8 was measured at 7.8e-2 - not viable).

Schedule: q_proj -> k_proj -> attention. The v projection's matmuls are
interleaved into attention half 0 pass A (heads 0,1 processed tk-major,
with v chunk tk emitted before the attnv tiles that read it), heads 2,3
run as pass B, and o_proj chunks m<8 are interleaved between half-1
heads; m>=8 trail (they need every head's half-1 output). PSUM stays
within 8 banks in every phase-overlap window.

Layouts (partition dim first):
  hT   [D, L] bf16              hidden[b].T, streamed [128,4,512] tiles
  wq/wk/wv packed [128,16,512]  (p,k,f) = w[r0+f, k*128+p]
  wo packed [128,4,2048]        (p,h,d) = wo[d, r0+h*128+p]
  qT/kT per head [Hd, L]        scores = kT_tile.T @ qT (transposed scores)
  v_big [tok128, 16, 512]       attn@v stationary tiles [tok,Hd]
  probs [tk=128, tq=1024] bf16  exp(scoresT) per (h, half, tk)
  outT per head [Hd, L] bf16    normalized attn out = o_proj stationary

Softmax: no max-subtraction (scores ~ N(0,1)); denominator = partition-sum
of probs, built as two bf16 partial accumulators on DVE (8 tiles each),
then one PE matmul against an all-ones [128,128] matrix which both sums
the partials in fp32 PSUM and broadcasts den across partitions.
1/den = exp(-ln(den)) on ACT (same activation-table set as Exp). ps_o is
evacuated to SBUF immediately (cheap DVE copy) so the den->ln->rec->mul
chain stays off the PSUM critical path.
"""

import sys
import types

import numpy as np
import ml_dtypes

# Defensive: concourse.bass_utils imports antenv.axon_hooks when tracing is
# requested; provide a null shim if the module is absent in this image.
try:
    import antenv.axon_hooks  # noqa: F401
except ImportError:
    _m = types.ModuleType("antenv.axon_hooks")
    _m.set_axon_ntff_profile_hook = lambda h: None
    _m.get_axon_ntff_profile_hook = lambda: None
    sys.modules["antenv.axon_hooks"] = _m

import concourse.mybir as mybir
import concourse.tile as tile
from concourse import bacc
from concourse.bass_utils import run_bass_kernel_spmd

# problem constants (hardcoded per spec)
B, L, D = 2, 2048, 2048
H, Hd = 16, 128
NC = 8
TPH = 4            # heads per core
QKV = TPH * Hd     # 512 per-core projection width
KT = D // 128      # 16 contraction tiles
NT = L // 512      # 4 token groups of 512
MT = L // 128      # 16 token chunks of 128
HW = 1024          # tq half-width in attention

f32 = mybir.dt.float32
bf16 = mybir.dt.bfloat16
BF = ml_dtypes.bfloat16

AF = mybir.ActivationFunctionType
SCALE = 1.0 / float(np.sqrt(Hd))

_CACHE: dict = {}


def _build():
    nc = bacc.Bacc("TRN2", target_bir_lowering=False, debug=False)

    # hT host-packed as [p, n, k, t']: one contiguous 4KB line per partition
    # per [128,4,512] stream tile (vs 4x1KB strided from a plain [D, L] layout)
    hT = nc.dram_tensor("hT", [128, NT * KT * 512], bf16, kind="ExternalInput").ap()
    wq_d = nc.dram_tensor("wq", [128, KT * QKV], bf16, kind="ExternalInput").ap()
    wk_d = nc.dram_tensor("wk", [128, KT * QKV], bf16, kind="ExternalInput").ap()
    wv_d = nc.dram_tensor("wv", [128, KT * QKV], bf16, kind="ExternalInput").ap()
    wo_d = nc.dram_tensor("wo", [128, TPH * D], bf16, kind="ExternalInput").ap()
    cos_d = nc.dram_tensor("cosT", [Hd, L], bf16, kind="ExternalInput").ap()
    sin_d = nc.dram_tensor("sinTs", [Hd, L], bf16, kind="ExternalInput").ap()
    rot_d = nc.dram_tensor("rotM", [Hd, Hd], bf16, kind="ExternalInput").ap()
    out = nc.dram_tensor("out", [L, D], bf16, kind="ExternalOutput").ap()

    hT4 = hT.rearrange("p (n k t) -> p n k t", n=NT, k=KT)  # [128, 4, 16, 512]
    wq3 = wq_d.rearrange("p (k n) -> p k n", k=KT)      # [128, 16, 512]
    wk3 = wk_d.rearrange("p (k n) -> p k n", k=KT)
    wv3 = wv_d.rearrange("p (k n) -> p k n", k=KT)
    wo3 = wo_d.rearrange("p (h d) -> p h d", h=TPH)     # [128, 4, 2048]
    out3 = out.rearrange("(mm p) d -> p mm d", p=128)   # [128, 16, 2048]

    with tile.TileContext(nc) as tc:
        with tc.tile_pool(name="persist", bufs=1) as persist:
            # ---- constants + persistent tensors ---------------------------
            ones_mat = persist.tile([128, 128], bf16, name="ones_mat")
            nc.vector.memset(ones_mat, 1.0)
            cos_sb = persist.tile([Hd, L], bf16, name="cos_sb")
            sin_sb = persist.tile([Hd, L], bf16, name="sin_sb")
            rot_sb = persist.tile([Hd, Hd], bf16, name="rot_sb")
            # off the hot sync/gpsimd queues - not needed until rope
            nc.scalar.dma_start(out=rot_sb, in_=rot_d)
            nc.scalar.dma_start(out=cos_sb, in_=cos_d)
            nc.scalar.dma_start(out=sin_sb, in_=sin_d)

            wq_sb = persist.tile([128, KT, QKV], bf16, name="wq_sb")
            wk_sb = persist.tile([128, KT, QKV], bf16, name="wk_sb")
            wv_sb = persist.tile([128, KT, QKV], bf16, name="wv_sb")
            wo_sb = persist.tile([128, TPH, D], bf16, name="wo_sb")
            # q first (it runs first); progressive chunks so matmuls start
            # early - the first contraction chunk even per-m so MM #1 only
            # waits on 32KB
            for m in range(TPH):
                msl = slice(m * 128, (m + 1) * 128)
                nc.gpsimd.dma_start(out=wq_sb[:, 0:1, msl], in_=wq3[:, 0:1, msl])
            for a, b in ((1, 2), (2, 4), (4, 8), (8, 16)):
                nc.gpsimd.dma_start(out=wq_sb[:, a:b, :], in_=wq3[:, a:b, :])
            for a, b in ((0, 4), (4, 8), (8, 16)):
                nc.gpsimd.dma_start(out=wk_sb[:, a:b, :], in_=wk3[:, a:b, :])
            for a, b in ((0, 4), (4, 8), (8, 16)):
                nc.gpsimd.dma_start(out=wv_sb[:, a:b, :], in_=wv3[:, a:b, :])
            for hh in range(TPH):
                nc.gpsimd.dma_start(out=wo_sb[:, hh, :], in_=wo3[:, hh, :])

            qT = [persist.tile([Hd, L], bf16, name=f"qT{m}") for m in range(TPH)]
            kT = [persist.tile([Hd, L], bf16, name=f"kT{m}") for m in range(TPH)]
            v_big = persist.tile([128, MT, QKV], bf16, name="v_big")
            outT = [persist.tile([Hd, L], bf16, name=f"outT{m}") for m in range(TPH)]



            # ---- phase 1: q and k projections + RoPE ----------------------
            with tc.tile_pool(name="stream", bufs=3) as stream, tc.tile_pool(
                name="ropep", bufs=3
            ) as ropep:
                first = True
                for wsb, dst, tag in ((wq_sb, qT, "q"), (wk_sb, kT, "k")):
                    with nc.named_scope(f"{tag}_proj"):
                        with (
                            tc.tile_pool(
                                name=f"psx_{tag}", bufs=1, space="PSUM"
                            ) as psx,
                            tc.tile_pool(
                                name=f"psr_{tag}", bufs=1, space="PSUM"
                            ) as psr,
                        ):
                            for n in range(NT):
                                csl = slice(n * 512, (n + 1) * 512)
                                ps_x = [
                                    psx.tile(
                                        [128, 512], f32, name=f"pp{m}",
                                        bufs=2 if m < 2 else 1,
                                    )
                                    for m in range(TPH)
                                ]
                                for kg in range(4):
                                    htk = stream.tile(
                                        [128, 4, 512], bf16, name="htk"
                                    )
                                    src = hT4[:, n, kg * 4 : (kg + 1) * 4, :]
                                    if first:
                                        # fine-grained so the first matmuls
                                        # can start as soon as 128KB lands
                                        for i in range(4):
                                            nc.sync.dma_start(
                                                out=htk[:, i, :],
                                                in_=src[:, i, :],
                                            )
                                    else:
                                        nc.sync.dma_start(out=htk, in_=src)
                                    for i in range(4):
                                        kk = kg * 4 + i
                                        st = dict(
                                            start=(kk == 0), stop=(kk == KT - 1)
                                        )
                                        for m in range(TPH):
                                            nc.tensor.matmul(
                                                ps_x[m],
                                                wsb[:, kk, m * 128 : (m + 1) * 128],
                                                htk[:, i, :],
                                                **st,
                                            )
                                first = False
                                for m in range(TPH):
                                    raw = ropep.tile(
                                        [128, 512], bf16, name="raw", bufs=3
                                    )
                                    nc.scalar.copy(raw, ps_x[m])
                                    ps_rot = psr.tile([128, 512], f32, name="ps_rot")
                                    nc.tensor.matmul(
                                        ps_rot, rot_sb, raw, start=True, stop=True
                                    )
                                    rotc = ropep.tile(
                                        [128, 512], bf16, name="rotc", bufs=3
                                    )
                                    nc.scalar.copy(rotc, ps_rot)
                                    t1 = ropep.tile(
                                        [128, 512], bf16, name="t1", bufs=2
                                    )
                                    nc.vector.tensor_mul(t1, raw, cos_sb[:, csl])
                                    t2 = ropep.tile(
                                        [128, 512], bf16, name="t2", bufs=2
                                    )
                                    nc.vector.tensor_mul(t2, rotc, sin_sb[:, csl])
                                    nc.vector.tensor_add(dst[m][:, csl], t1, t2)

            # ---- phase 2+3+4: attention with v_proj (half 0) and o_proj ---
            # (half 1) interleaved into the PE slack of the exp-paced stream
            with (
                tc.tile_pool(name="stream2", bufs=6) as stream2,
                tc.tile_pool(name="probsp", bufs=6) as probsp,
                tc.tile_pool(name="accp", bufs=1) as accp,
                tc.tile_pool(name="orawp", bufs=3) as orawp,
                tc.tile_pool(name="lnp", bufs=1) as lnp,
                tc.tile_pool(name="recp", bufs=2) as recp,
                tc.tile_pool(name="otp", bufs=2) as otp,
            ):

                def emit_tile(h, half, tk, ps_o, accs, pss):
                    """One (head, tq-half, key-chunk) attention tile."""
                    ps_sc = pss.tile([128, HW], f32, name="sc")
                    for j in range(2):
                        tq0 = half * HW + j * 512
                        nc.tensor.matmul(
                            ps_sc[:, j * 512 : (j + 1) * 512],
                            kT[h][:, tk * 128 : (tk + 1) * 128],
                            qT[h][:, tq0 : tq0 + 512],
                            start=True, stop=True,
                        )
                    probs = probsp.tile([128, HW], bf16, name="probs", bufs=6)
                    nc.scalar.activation(probs, ps_sc, AF.Exp, scale=SCALE)
                    st = dict(start=(tk == 0), stop=(tk == MT - 1))
                    for j in range(2):
                        nc.tensor.matmul(
                            ps_o[:, j * 512 : (j + 1) * 512],
                            v_big[:, tk, h * 128 : (h + 1) * 128],
                            probs[:, j * 512 : (j + 1) * 512],
                            **st,
                        )
                    g = tk // 8
                    if tk % 8 == 0:
                        acc = accp.tile([128, HW], bf16, name=f"acc{g}_{h & 1}")
                        nc.vector.tensor_copy(acc, probs)
                        accs.append(acc)
                    else:
                        nc.vector.tensor_add(accs[g], accs[g], probs)

                def emit_tail(h, half, ps_o, accs, den_pool, den_tag):
                    """Evacuate ps_o fast; den/recip/normalize off the PSUM
                    critical path. den_bc draws from den_pool/den_tag."""
                    sl = slice(half * HW, (half + 1) * HW)
                    oraw = orawp.tile([Hd, HW], bf16, name="oraw")
                    nc.vector.tensor_copy(oraw, ps_o)
                    den_bc = den_pool.tile([128, HW], f32, name=den_tag)
                    for j in range(2):
                        jsl = slice(j * 512, (j + 1) * 512)
                        nc.tensor.matmul(
                            den_bc[:, jsl], ones_mat, accs[0][:, jsl],
                            start=True, stop=False,
                        )
                        nc.tensor.matmul(
                            den_bc[:, jsl], ones_mat, accs[1][:, jsl],
                            start=False, stop=True,
                        )
                    ln_den = lnp.tile([128, HW], f32, name="ln_den")
                    nc.scalar.activation(ln_den, den_bc, AF.Ln)
                    rec_bc = recp.tile([128, HW], bf16, name="rec_bc")
                    nc.scalar.activation(rec_bc, ln_den, AF.Exp, scale=-1.0)
                    nc.vector.tensor_mul(outT[h][:, sl], oraw, rec_bc)

                with nc.named_scope("attention"):
                    # -- half 0, pass A: heads 0,1 tk-major with the v
                    # projection interleaved (chunk tk emitted before the
                    # tiles that read it)
                    with tc.tile_pool(name="ps_oA", bufs=1, space="PSUM") as psoA:
                        with (
                            tc.tile_pool(name="ps_v", bufs=2, space="PSUM") as psv,
                            tc.tile_pool(name="sc_a", bufs=1, space="PSUM") as scA,
                        ):
                            ps_os = {
                                h: psoA.tile([Hd, HW], f32, name=f"ps_o{h}")
                                for h in (0, 1)
                            }
                            accs = {0: [], 1: []}
                            htks = []
                            for tk in range(MT):
                                if tk % 4 == 0:
                                    n = tk // 4
                                    htks = []
                                    # first group on the idle gpsimd queue so
                                    # it overlaps k_proj's sync-queue stream
                                    dma_eng = nc.gpsimd if n == 0 else nc.sync
                                    for kg in range(4):
                                        htk2 = stream2.tile(
                                            [128, 4, 512], bf16, name="htk2"
                                        )
                                        dma_eng.dma_start(
                                            out=htk2,
                                            in_=hT4[:, n,
                                                    kg * 4 : (kg + 1) * 4, :],
                                        )
                                        htks.append(htk2)
                                ps_v = psv.tile([128, QKV], f32, name="ps_v")
                                for kg in range(4):
                                    for i in range(4):
                                        kk = kg * 4 + i
                                        nc.tensor.matmul(
                                            ps_v,
                                            htks[kg][:, i,
                                                     (tk % 4) * 128 :
                                                     (tk % 4 + 1) * 128],
                                            wv_sb[:, kk, :],
                                            start=(kk == 0), stop=(kk == KT - 1),
                                        )
                                nc.vector.tensor_copy(v_big[:, tk, :], ps_v)
                                for h in (0, 1):
                                    emit_tile(h, 0, tk, ps_os[h], accs[h], scA)
                            for h in (0, 1):
                                emit_tail(h, 0, ps_os[h], accs[h],
                                          psoA, f"ps_o{h}")

                        # -- half 0, pass B: heads 2,3 tk-major (ACT-paced)
                        with tc.tile_pool(name="sc_b", bufs=2, space="PSUM") as scB:
                            ps_os = {
                                h: psoA.tile([Hd, HW], f32, name=f"ps_o{h - 2}")
                                for h in (2, 3)
                            }
                            accs = {2: [], 3: []}
                            for tk in range(MT):
                                for h in (2, 3):
                                    emit_tile(h, 0, tk, ps_os[h], accs[h], scB)
                            for h in (2, 3):
                                emit_tail(h, 0, ps_os[h], accs[h],
                                          psoA, f"ps_o{h - 2}")

                    # -- half 1: heads sequential, o_proj interleaved
                    with nc.named_scope("o_proj"):
                        with (
                            tc.tile_pool(name="sc_c", bufs=2, space="PSUM") as scC,
                            tc.tile_pool(name="ps_oB", bufs=1, space="PSUM") as psoB,
                            tc.tile_pool(name="ps_f", bufs=2, space="PSUM") as psf,
                        ):

                            def emit_oproj(m):
                                ot = otp.tile([128, D], bf16, name="ot")
                                for np_ in range(2):
                                    ps_f = [
                                        psf.tile([128, 512], f32, name="ps_f")
                                        for _ in range(2)
                                    ]
                                    for h in range(TPH):
                                        for nn in range(2):
                                            n = np_ * 2 + nn
                                            nc.tensor.matmul(
                                                ps_f[nn],
                                                outT[h][:, m * 128 : (m + 1) * 128],
                                                wo_sb[:, h, n * 512 : (n + 1) * 512],
                                                start=(h == 0), stop=(h == TPH - 1),
                                            )
                                    for nn in range(2):
                                        n = np_ * 2 + nn
                                        nc.vector.tensor_copy(
                                            ot[:, n * 512 : (n + 1) * 512], ps_f[nn]
                                        )
                                nc.sync.dma_start(out=out3[:, m, :], in_=ot)

                            for h in range(TPH):
                                ps_o = psoB.tile([Hd, HW], f32, name="ps_o")
                                accs = []
                                for tk in range(MT):
                                    emit_tile(h, 1, tk, ps_o, accs, scC)
                                emit_tail(h, 1, ps_o, accs, scC, "sc")
                                emit_oproj(2 * h)
                                emit_oproj(2 * h + 1)
                            for m in range(8, MT):
                                emit_oproj(m)

    nc.compile()
    return nc


def kernel(hidden_states, cos, sin, wq, wk, wv, wo):
    if "nc" not in _CACHE:
        _CACHE["nc"] = _build()
    nc = _CACHE["nc"]

    hidden_states = np.asarray(hidden_states, dtype=np.float32)
    cos = np.asarray(cos, dtype=np.float32)
    sin = np.asarray(sin, dtype=np.float32)
    wq = np.asarray(wq, dtype=np.float32)
    wk = np.asarray(wk, dtype=np.float32)
    wv = np.asarray(wv, dtype=np.float32)
    wo = np.asarray(wo, dtype=np.float32)

    # host-side layout prep
    cosT = np.ascontiguousarray(cos[0, 0].T).astype(BF)      # [Hd, L]
    sinT = np.ascontiguousarray(sin[0, 0].T)
    sinTs = sinT.copy()
    sinTs[: Hd // 2] *= -1.0                                 # fold rotate_half signs
    sinTs = sinTs.astype(BF)
    rot = np.zeros((Hd, Hd), dtype=np.float32)               # pure half-swap perm
    for p in range(Hd // 2):
        rot[p, p + Hd // 2] = 1.0
        rot[p + Hd // 2, p] = 1.0
    rotM = rot.astype(BF)

    # pack hT as [p, n, k, t']: element = hidden[b][n*512+t', k*128+p]
    hT = [
        np.ascontiguousarray(
            hidden_states[b].reshape(NT, 512, KT, 128).transpose(3, 0, 2, 1)
        ).astype(BF).reshape(128, NT * KT * 512)
        for b in range(B)
    ]

    def pack_qkv(w, r0):
        # [128, 16*512]: (p, k, f) = w[r0+f, k*128+p]
        wc = w[r0 : r0 + QKV].T                              # [2048, 512]
        a = wc.reshape(KT, 128, QKV).transpose(1, 0, 2)      # [128, 16, 512]
        return np.ascontiguousarray(a).astype(BF).reshape(128, KT * QKV)

    def pack_wo(w, r0):
        # [128, 4*2048]: (p, h, d) = wo[d, r0+h*128+p]
        wc = w[:, r0 : r0 + QKV].T                           # [512, 2048]
        a = wc.reshape(TPH, 128, D).transpose(1, 0, 2)       # [128, 4, 2048]
        return np.ascontiguousarray(a).astype(BF).reshape(128, TPH * D)

    in_maps = []
    for c in range(NC):
        b = c // 4
        r0 = (c % 4) * QKV
        in_maps.append(
            {
                "hT": hT[b],
                "wq": pack_qkv(wq, r0),
                "wk": pack_qkv(wk, r0),
                "wv": pack_qkv(wv, r0),
                "wo": pack_wo(wo, r0),
                "cosT": cosT,
                "sinTs": sinTs,
                "rotM": rotM,
            }
        )

    res = run_bass_kernel_spmd(nc, in_maps, core_ids=list(range(NC)))
    _CACHE["last_results"] = res

    out = np.zeros((B, L, D), dtype=np.float32)
    for c in range(NC):
        out[c // 4] += np.asarray(res.results[c]["out"]).astype(np.float32)
    return out


# revision 32
# speedup vs baseline: 1.1849x; 1.1849x over previous
"""Multi-head attention (QKV proj + RoPE + SDPA + o_proj) on 8 TRN2 cores.

Sharding: DP2 x TP4. Core c handles batch c//4 and heads 4*(c%4)..4*(c%4)+3.
Each core computes its 4 heads' attention and a partial o_proj output
[L, D]; the host sums the 4 partials per batch (row-parallel o_proj).

All matmul operands are bf16 (full-rate on the PE, half the DMA/SBUF of
fp32); accumulation stays fp32 in PSUM. rel-err vs the fp32 reference is
~1e-2, within the 2e-2 gate (fp8 was measured at 7.8e-2 - not viable).

Schedule: q_proj -> k_proj -> attention. The v projection's matmuls are
interleaved into attention half 0 pass A (heads 0,1 processed tk-major,
with v chunk tk emitted before the attnv tiles that read it), heads 2,3
run as pass B, and o_proj chunks m<8 are interleaved between half-1
heads; m>=8 trail (they need every head's half-1 output). PSUM stays
within 8 banks in every phase-overlap window.

Layouts (partition dim first):
  hT   [D, L] bf16              hidden[b].T, streamed [128,4,512] tiles
  wq/wk/wv packed [128,16,512]  (p,k,f) = w[r0+f, k*128+p]
  wo packed [128,4,2048]        (p,h,d) = wo[d, r0+h*128+p]
  qT/kT per head [Hd, L]        scores = kT_tile.T @ qT (transposed scores)
  v_big [tok128, 16, 512]       attn@v stationary tiles [tok,Hd]
  probs [tk=128, tq=1024] bf16  exp(scoresT) per (h, half, tk)
  outT per head [Hd, L] bf16    normalized attn out = o_proj stationary

Softmax: no max-subtraction (scores ~ N(0,1)); denominator = partition-sum
of probs, built as two bf16 partial accumulators on DVE (8 tiles each),
then one PE matmul against an all-ones [128,128] matrix which both sums
the partials in fp32 PSUM and broadcasts den across partitions.
1/den = exp(-ln(den)) on ACT (same activation-table set as Exp). ps_o is
evacuated to SBUF immediately (cheap DVE copy) so the den->ln->rec->mul
chain stays off the PSUM critical path.
"""

import sys
import types

import numpy as np
import ml_dtypes

# Defensive: concourse.bass_utils imports antenv.axon_hooks when tracing is
# requested; provide a null shim if the module is absent in this image.
try:
    import antenv.axon_hooks  # noqa: F401
except ImportError:
    _m = types.ModuleType("antenv.axon_hooks")
    _m.set_axon_ntff_profile_hook = lambda h: None
    _m.get_axon_ntff_profile_hook = lambda: None
    sys.modules["antenv.axon_hooks"] = _m

import concourse.mybir as mybir
import concourse.tile as tile
from concourse import bacc
from concourse.bass_utils import run_bass_kernel_spmd

# problem constants (hardcoded per spec)
B, L, D = 2, 2048, 2048
H, Hd = 16, 128
NC = 8
TPH = 4            # heads per core
QKV = TPH * Hd     # 512 per-core projection width
KT = D // 128      # 16 contraction tiles
NT = L // 512      # 4 token groups of 512
MT = L // 128      # 16 token chunks of 128
HW = 1024          # tq half-width in attention

f32 = mybir.dt.float32
bf16 = mybir.dt.bfloat16
BF = ml_dtypes.bfloat16

AF = mybir.ActivationFunctionType
SCALE = 1.0 / float(np.sqrt(Hd))

_CACHE: dict = {}


def _build():
    nc = bacc.Bacc("TRN2", target_bir_lowering=False, debug=False)

    # hT host-packed as [p, n, k, t']: one contiguous 4KB line per partition
    # per [128,4,512] stream tile (vs 4x1KB strided from a plain [D, L] layout)
    hT = nc.dram_tensor("hT", [128, NT * KT * 512], bf16, kind="ExternalInput").ap()
    wq_d = nc.dram_tensor("wq", [128, KT * QKV], bf16, kind="ExternalInput").ap()
    wk_d = nc.dram_tensor("wk", [128, KT * QKV], bf16, kind="ExternalInput").ap()
    wv_d = nc.dram_tensor("wv", [128, KT * QKV], bf16, kind="ExternalInput").ap()
    wo_d = nc.dram_tensor("wo", [128, TPH * D], bf16, kind="ExternalInput").ap()
    cos_d = nc.dram_tensor("cosT", [Hd, L], bf16, kind="ExternalInput").ap()
    sin_d = nc.dram_tensor("sinTs", [Hd, L], bf16, kind="ExternalInput").ap()
    rot_d = nc.dram_tensor("rotM", [Hd, Hd], bf16, kind="ExternalInput").ap()
    out = nc.dram_tensor("out", [L, D], bf16, kind="ExternalOutput").ap()

    hT4 = hT.rearrange("p (n k t) -> p n k t", n=NT, k=KT)  # [128, 4, 16, 512]
    wq3 = wq_d.rearrange("p (k n) -> p k n", k=KT)      # [128, 16, 512]
    wk3 = wk_d.rearrange("p (k n) -> p k n", k=KT)
    wv3 = wv_d.rearrange("p (k n) -> p k n", k=KT)
    wo3 = wo_d.rearrange("p (h d) -> p h d", h=TPH)     # [128, 4, 2048]
    out3 = out.rearrange("(mm p) d -> p mm d", p=128)   # [128, 16, 2048]

    with tile.TileContext(nc) as tc:
        with tc.tile_pool(name="persist", bufs=1) as persist:
            # ---- constants + persistent tensors ---------------------------
            ones_mat = persist.tile([128, 128], bf16, name="ones_mat")
            nc.vector.memset(ones_mat, 1.0)
            cos_sb = persist.tile([Hd, L], bf16, name="cos_sb")
            sin_sb = persist.tile([Hd, L], bf16, name="sin_sb")
            rot_sb = persist.tile([Hd, Hd], bf16, name="rot_sb")
            # off the hot sync/gpsimd queues - not needed until rope
            nc.scalar.dma_start(out=rot_sb, in_=rot_d)
            nc.scalar.dma_start(out=cos_sb, in_=cos_d)
            nc.scalar.dma_start(out=sin_sb, in_=sin_d)

            wq_sb = persist.tile([128, KT, QKV], bf16, name="wq_sb")
            wk_sb = persist.tile([128, KT, QKV], bf16, name="wk_sb")
            wv_sb = persist.tile([128, KT, QKV], bf16, name="wv_sb")
            wo_sb = persist.tile([128, TPH, D], bf16, name="wo_sb")
            # q first (it runs first); progressive chunks so matmuls start
            # early - the first contraction chunk even per-m so MM #1 only
            # waits on 32KB
            for m in range(TPH):
                msl = slice(m * 128, (m + 1) * 128)
                nc.gpsimd.dma_start(out=wq_sb[:, 0:1, msl], in_=wq3[:, 0:1, msl])
            for a, b in ((1, 2), (2, 4), (4, 8), (8, 16)):
                nc.gpsimd.dma_start(out=wq_sb[:, a:b, :], in_=wq3[:, a:b, :])
            for a, b in ((0, 4), (4, 8), (8, 16)):
                nc.gpsimd.dma_start(out=wk_sb[:, a:b, :], in_=wk3[:, a:b, :])
            for a, b in ((0, 4), (4, 8), (8, 16)):
                nc.gpsimd.dma_start(out=wv_sb[:, a:b, :], in_=wv3[:, a:b, :])
            for hh in range(TPH):
                nc.gpsimd.dma_start(out=wo_sb[:, hh, :], in_=wo3[:, hh, :])

            qT = [persist.tile([Hd, L], bf16, name=f"qT{m}") for m in range(TPH)]
            kT = [persist.tile([Hd, L], bf16, name=f"kT{m}") for m in range(TPH)]
            v_big = persist.tile([128, MT, QKV], bf16, name="v_big")
            outT = [persist.tile([Hd, L], bf16, name=f"outT{m}") for m in range(TPH)]



            # ---- phase 1: q and k projections + RoPE ----------------------
            with tc.tile_pool(name="stream", bufs=3) as stream, tc.tile_pool(
                name="ropep", bufs=3
            ) as ropep:
                first = True
                for wsb, dst, tag in ((wq_sb, qT, "q"), (wk_sb, kT, "k")):
                    with nc.named_scope(f"{tag}_proj"):
                        with (
                            tc.tile_pool(
                                name=f"psx_{tag}", bufs=1, space="PSUM"
                            ) as psx,
                            tc.tile_pool(
                                name=f"psr_{tag}", bufs=2, space="PSUM"
                            ) as psr,
                        ):
                            for n in range(NT):
                                csl = slice(n * 512, (n + 1) * 512)
                                ps_x = [
                                    psx.tile(
                                        [128, 512], f32, name=f"pp{m}",
                                        bufs=2 if m < 2 else 1,
                                    )
                                    for m in range(TPH)
                                ]
                                for kg in range(4):
                                    htk = stream.tile(
                                        [128, 4, 512], bf16, name="htk"
                                    )
                                    src = hT4[:, n, kg * 4 : (kg + 1) * 4, :]
                                    if first:
                                        # fine-grained so the first matmuls
                                        # can start as soon as 128KB lands
                                        for i in range(4):
                                            nc.sync.dma_start(
                                                out=htk[:, i, :],
                                                in_=src[:, i, :],
                                            )
                                    else:
                                        nc.sync.dma_start(out=htk, in_=src)
                                    for i in range(4):
                                        kk = kg * 4 + i
                                        st = dict(
                                            start=(kk == 0), stop=(kk == KT - 1)
                                        )
                                        for m in range(TPH):
                                            nc.tensor.matmul(
                                                ps_x[m],
                                                wsb[:, kk, m * 128 : (m + 1) * 128],
                                                htk[:, i, :],
                                                **st,
                                            )
                                first = False
                                for m in range(TPH):
                                    raw = ropep.tile(
                                        [128, 512], bf16, name="raw", bufs=3
                                    )
                                    nc.scalar.copy(raw, ps_x[m])
                                    ps_rot = psr.tile([128, 512], f32, name="ps_rot")
                                    nc.tensor.matmul(
                                        ps_rot, rot_sb, raw, start=True, stop=True
                                    )
                                    rotc = ropep.tile(
                                        [128, 512], bf16, name="rotc", bufs=3
                                    )
                                    nc.scalar.copy(rotc, ps_rot)
                                    t1 = ropep.tile(
                                        [128, 512], bf16, name="t1", bufs=2
                                    )
                                    nc.vector.tensor_mul(t1, raw, cos_sb[:, csl])
                                    t2 = ropep.tile(
                                        [128, 512], bf16, name="t2", bufs=2
                                    )
                                    nc.vector.tensor_mul(t2, rotc, sin_sb[:, csl])
                                    nc.vector.tensor_add(dst[m][:, csl], t1, t2)

            # ---- phase 2+3+4: attention with v_proj (half 0) and o_proj ---
            # (half 1) interleaved into the PE slack of the exp-paced stream
            with (
                tc.tile_pool(name="stream2", bufs=6) as stream2,
                tc.tile_pool(name="probsp", bufs=6) as probsp,
                tc.tile_pool(name="accp", bufs=1) as accp,
                tc.tile_pool(name="orawp", bufs=3) as orawp,
                tc.tile_pool(name="lnp", bufs=1) as lnp,
                tc.tile_pool(name="recp", bufs=2) as recp,
                tc.tile_pool(name="otp", bufs=2) as otp,
            ):

                def emit_tile(h, half, tk, ps_o, accs, pss):
                    """One (head, tq-half, key-chunk) attention tile."""
                    ps_sc = pss.tile([128, HW], f32, name="sc")
                    for j in range(2):
                        tq0 = half * HW + j * 512
                        nc.tensor.matmul(
                            ps_sc[:, j * 512 : (j + 1) * 512],
                            kT[h][:, tk * 128 : (tk + 1) * 128],
                            qT[h][:, tq0 : tq0 + 512],
                            start=True, stop=True,
                        )
                    probs = probsp.tile([128, HW], bf16, name="probs", bufs=6)
                    nc.scalar.activation(probs, ps_sc, AF.Exp, scale=SCALE)
                    st = dict(start=(tk == 0), stop=(tk == MT - 1))
                    for j in range(2):
                        nc.tensor.matmul(
                            ps_o[:, j * 512 : (j + 1) * 512],
                            v_big[:, tk, h * 128 : (h + 1) * 128],
                            probs[:, j * 512 : (j + 1) * 512],
                            **st,
                        )
                    g = tk // 8
                    if tk % 8 == 0:
                        acc = accp.tile([128, HW], bf16, name=f"acc{g}_{h & 1}")
                        nc.vector.tensor_copy(acc, probs)
                        accs.append(acc)
                    else:
                        nc.vector.tensor_add(accs[g], accs[g], probs)

                def emit_tail(h, half, ps_o, accs, den_pool, den_tag):
                    """Evacuate ps_o fast; den/recip/normalize off the PSUM
                    critical path. den_bc draws from den_pool/den_tag."""
                    sl = slice(half * HW, (half + 1) * HW)
                    oraw = orawp.tile([Hd, HW], bf16, name="oraw")
                    nc.vector.tensor_copy(oraw, ps_o)
                    den_bc = den_pool.tile([128, HW], f32, name=den_tag)
                    for j in range(2):
                        jsl = slice(j * 512, (j + 1) * 512)
                        nc.tensor.matmul(
                            den_bc[:, jsl], ones_mat, accs[0][:, jsl],
                            start=True, stop=False,
                        )
                        nc.tensor.matmul(
                            den_bc[:, jsl], ones_mat, accs[1][:, jsl],
                            start=False, stop=True,
                        )
                    ln_den = lnp.tile([128, HW], f32, name="ln_den")
                    nc.scalar.activation(ln_den, den_bc, AF.Ln)
                    rec_bc = recp.tile([128, HW], bf16, name="rec_bc")
                    nc.scalar.activation(rec_bc, ln_den, AF.Exp, scale=-1.0)
                    nc.vector.tensor_mul(outT[h][:, sl], oraw, rec_bc)

                with nc.named_scope("attention"):
                    # -- half 0, pass A: heads 0,1 tk-major with the v
                    # projection interleaved (chunk tk emitted before the
                    # tiles that read it)
                    with tc.tile_pool(name="ps_oA", bufs=1, space="PSUM") as psoA:
                        with (
                            tc.tile_pool(name="ps_v", bufs=2, space="PSUM") as psv,
                            tc.tile_pool(name="sc_a", bufs=1, space="PSUM") as scA,
                        ):
                            ps_os = {
                                h: psoA.tile([Hd, HW], f32, name=f"ps_o{h}")
                                for h in (0, 1)
                            }
                            accs = {0: [], 1: []}
                            htks = []
                            for tk in range(MT):
                                if tk % 4 == 0:
                                    n = tk // 4
                                    htks = []
                                    # first group on the idle gpsimd queue so
                                    # it overlaps k_proj's sync-queue stream
                                    dma_eng = nc.gpsimd if n == 0 else nc.sync
                                    for kg in range(4):
                                        htk2 = stream2.tile(
                                            [128, 4, 512], bf16, name="htk2"
                                        )
                                        dma_eng.dma_start(
                                            out=htk2,
                                            in_=hT4[:, n,
                                                    kg * 4 : (kg + 1) * 4, :],
                                        )
                                        htks.append(htk2)
                                ps_v = psv.tile([128, QKV], f32, name="ps_v")
                                for kg in range(4):
                                    for i in range(4):
                                        kk = kg * 4 + i
                                        nc.tensor.matmul(
                                            ps_v,
                                            htks[kg][:, i,
                                                     (tk % 4) * 128 :
                                                     (tk % 4 + 1) * 128],
                                            wv_sb[:, kk, :],
                                            start=(kk == 0), stop=(kk == KT - 1),
                                        )
                                nc.vector.tensor_copy(v_big[:, tk, :], ps_v)
                                for h in (0, 1):
                                    emit_tile(h, 0, tk, ps_os[h], accs[h], scA)
                            for h in (0, 1):
                                emit_tail(h, 0, ps_os[h], accs[h],
                                          psoA, f"ps_o{h}")

                        # -- half 0, pass B: heads 2,3 tk-major (ACT-paced)
                        with tc.tile_pool(name="sc_b", bufs=2, space="PSUM") as scB:
                            ps_os = {
                                h: psoA.tile([Hd, HW], f32, name=f"ps_o{h - 2}")
                                for h in (2, 3)
                            }
                            accs = {2: [], 3: []}
                            for tk in range(MT):
                                for h in (2, 3):
                                    emit_tile(h, 0, tk, ps_os[h], accs[h], scB)
                            for h in (2, 3):
                                emit_tail(h, 0, ps_os[h], accs[h],
                                          psoA, f"ps_o{h - 2}")

                    # -- half 1: heads sequential, o_proj interleaved
                    with nc.named_scope("o_proj"):
                        with (
                            tc.tile_pool(name="sc_c", bufs=2, space="PSUM") as scC,
                            tc.tile_pool(name="ps_oB", bufs=1, space="PSUM") as psoB,
                            tc.tile_pool(name="ps_f", bufs=2, space="PSUM") as psf,
                        ):

                            def emit_oproj(m):
                                ot = otp.tile([128, D], bf16, name="ot")
                                for np_ in range(2):
                                    ps_f = [
                                        psf.tile([128, 512], f32, name="ps_f")
                                        for _ in range(2)
                                    ]
                                    for h in range(TPH):
                                        for nn in range(2):
                                            n = np_ * 2 + nn
                                            nc.tensor.matmul(
                                                ps_f[nn],
                                                outT[h][:, m * 128 : (m + 1) * 128],
                                                wo_sb[:, h, n * 512 : (n + 1) * 512],
                                                start=(h == 0), stop=(h == TPH - 1),
                                            )
                                    for nn in range(2):
                                        n = np_ * 2 + nn
                                        nc.vector.tensor_copy(
                                            ot[:, n * 512 : (n + 1) * 512], ps_f[nn]
                                        )
                                nc.sync.dma_start(out=out3[:, m, :], in_=ot)

                            for h in range(TPH):
                                ps_o = psoB.tile([Hd, HW], f32, name="ps_o")
                                accs = []
                                for tk in range(MT):
                                    emit_tile(h, 1, tk, ps_o, accs, scC)
                                emit_tail(h, 1, ps_o, accs, scC, "sc")
                                emit_oproj(2 * h)
                                emit_oproj(2 * h + 1)
                            for m in range(8, MT):
                                emit_oproj(m)

    nc.compile()
    return nc


def kernel(hidden_states, cos, sin, wq, wk, wv, wo):
    if "nc" not in _CACHE:
        _CACHE["nc"] = _build()
    nc = _CACHE["nc"]

    hidden_states = np.asarray(hidden_states, dtype=np.float32)
    cos = np.asarray(cos, dtype=np.float32)
    sin = np.asarray(sin, dtype=np.float32)
    wq = np.asarray(wq, dtype=np.float32)
    wk = np.asarray(wk, dtype=np.float32)
    wv = np.asarray(wv, dtype=np.float32)
    wo = np.asarray(wo, dtype=np.float32)

    # host-side layout prep
    cosT = np.ascontiguousarray(cos[0, 0].T).astype(BF)      # [Hd, L]
    sinT = np.ascontiguousarray(sin[0, 0].T)
    sinTs = sinT.copy()
    sinTs[: Hd // 2] *= -1.0                                 # fold rotate_half signs
    sinTs = sinTs.astype(BF)
    rot = np.zeros((Hd, Hd), dtype=np.float32)               # pure half-swap perm
    for p in range(Hd // 2):
        rot[p, p + Hd // 2] = 1.0
        rot[p + Hd // 2, p] = 1.0
    rotM = rot.astype(BF)

    # pack hT as [p, n, k, t']: element = hidden[b][n*512+t', k*128+p]
    hT = [
        np.ascontiguousarray(
            hidden_states[b].reshape(NT, 512, KT, 128).transpose(3, 0, 2, 1)
        ).astype(BF).reshape(128, NT * KT * 512)
        for b in range(B)
    ]

    def pack_qkv(w, r0):
        # [128, 16*512]: (p, k, f) = w[r0+f, k*128+p]
        wc = w[r0 : r0 + QKV].T                              # [2048, 512]
        a = wc.reshape(KT, 128, QKV).transpose(1, 0, 2)      # [128, 16, 512]
        return np.ascontiguousarray(a).astype(BF).reshape(128, KT * QKV)

    def pack_wo(w, r0):
        # [128, 4*2048]: (p, h, d) = wo[d, r0+h*128+p]
        wc = w[:, r0 : r0 + QKV].T                           # [512, 2048]
        a = wc.reshape(TPH, 128, D).transpose(1, 0, 2)       # [128, 4, 2048]
        return np.ascontiguousarray(a).astype(BF).reshape(128, TPH * D)

    in_maps = []
    for c in range(NC):
        b = c // 4
        r0 = (c % 4) * QKV
        in_maps.append(
            {
                "hT": hT[b],
                "wq": pack_qkv(wq, r0),
                "wk": pack_qkv(wk, r0),
                "wv": pack_qkv(wv, r0),
                "wo": pack_wo(wo, r0),
                "cosT": cosT,
                "sinTs": sinTs,
                "rotM": rotM,
            }
        )

    res = run_bass_kernel_spmd(nc, in_maps, core_ids=list(range(NC)))
    _CACHE["last_results"] = res

    out = np.zeros((B, L, D), dtype=np.float32)
    for c in range(NC):
        out[c // 4] += np.asarray(res.results[c]["out"]).astype(np.float32)
    return out
